# revision 3
# baseline (speedup 1.0000x reference)
"""Trainium2 Bass kernel for nn_DecoderModule (topk_masking).

Strategy: data-parallel over num_hyps across 8 NeuronCores. Each core
computes, for its 8192-hyp shard, per-row softmax statistics
(sumexp(logits) and max(exp(logits))) of the joiner logits. The host
then ranks rows by rowM = hyps_log_prob + log(max_exp) - log(sum_exp)
(exactly the per-row max of the final log-probs), recomputes the top
candidate rows exactly in f32, and takes the global top-k. This is the
"per-shard top-k + all-gather + global top-k" scheme with the per-shard
top-k expressed as per-row stats (a row can contribute up to beam=4
candidates, so the top-4 rows by row-max are a guaranteed superset).

Device pipeline per 128-hyp tile:
  - dma_gather(transpose=True) pulls conv-folded embedding table rows
    (T0[tok0], T1[tok1]) already transposed to feature-major bf16
  - DVE add + relu -> decT (bf16)
  - 16 bf16 matmuls: PT = proj_w^T-chunks @ decT-chunks (feature-major)
  - DVE add with host-pretransposed encoder (proj_b folded in)
  - ScalarE tanh -> AT
  - 4 f32r matmuls + 1 bias matmul -> logits (PSUM)
  - ScalarE Exp with accum_out -> sumexp per row; DVE reduce-max of exp
"""

import numpy as np

NUM_HYPS = 65536
VOCAB = 500
DEC_DIM = 512
JOINER_DIM = 512
CTX = 2
NCORES = 8
NLOC = NUM_HYPS // NCORES          # 8192 hyps per core
NT = NLOC // 128                   # 64 tiles per core
TOPROWS = 16                       # rows recomputed exactly on host

_CACHE = {}


def _build_program(debug_tile=None):
    import concourse.bacc as bacc
    import concourse.mybir as mybir
    from concourse.tile import TileContext
    from concourse.bass import ts

    dt = mybir.dt
    nc = bacc.Bacc("TRN2", debug=False, num_devices=NCORES)

    t0_d = nc.dram_tensor("t0", [VOCAB, DEC_DIM], dt.bfloat16, kind="ExternalInput")
    t1_d = nc.dram_tensor("t1", [VOCAB, DEC_DIM], dt.bfloat16, kind="ExternalInput")
    encT_d = nc.dram_tensor("encT", [4, 128, NLOC], dt.float32, kind="ExternalInput")
    idx_d = nc.dram_tensor("idx", [128, NT * 16], dt.int16, kind="ExternalInput")
    pwT_d = nc.dram_tensor("pwT", [128, 4 * 512], dt.bfloat16, kind="ExternalInput")
    jwT_d = nc.dram_tensor("jwT", [128, 4 * 500], dt.float32r, kind="ExternalInput")
    jb_d = nc.dram_tensor("jb", [1, 500], dt.float32r, kind="ExternalInput")
    ones_d = nc.dram_tensor("ones", [1, 128], dt.float32r, kind="ExternalInput")
    s_d = nc.dram_tensor("s_out", [128, NT], dt.float32, kind="ExternalOutput")
    em_d = nc.dram_tensor("em_out", [128, NT], dt.float32, kind="ExternalOutput")
    dbg = {}
    if debug_tile is not None:
        dbg["decT"] = nc.dram_tensor("dbg_decT", [128, 512], dt.bfloat16, kind="ExternalOutput")
        dbg["at"] = nc.dram_tensor("dbg_at", [128, 512], dt.float32, kind="ExternalOutput")
        dbg["logits"] = nc.dram_tensor("dbg_logits", [128, 500], dt.float32, kind="ExternalOutput")

    f32r = dt.float32r

    with TileContext(nc) as tc:
        with (
            tc.tile_pool(name="consts", bufs=1) as cpool,
            tc.tile_pool(name="enc", bufs=3) as enc_pool,
            tc.tile_pool(name="gather", bufs=3) as g_pool,
            tc.tile_pool(name="work", bufs=3) as w_pool,
            tc.tile_pool(name="psum_pt", bufs=2, space="PSUM") as pt_pool,
            tc.tile_pool(name="psum_lg", bufs=2, space="PSUM") as lg_pool,
        ):
            idx_sb = cpool.tile([128, NT * 16], dt.int16)
            nc.gpsimd.dma_start(idx_sb[:], idx_d[:])
            pwT_sb = cpool.tile([128, 4 * 512], dt.bfloat16)
            nc.gpsimd.dma_start(pwT_sb[:], pwT_d[:])
            jwT_sb = cpool.tile([128, 4 * 500], dt.float32r)
            nc.gpsimd.dma_start(jwT_sb[:], jwT_d[:])
            jb_sb = cpool.tile([1, 500], dt.float32r)
            nc.gpsimd.dma_start(jb_sb[:], jb_d[:])
            ones_sb = cpool.tile([1, 128], dt.float32r)
            nc.gpsimd.dma_start(ones_sb[:], ones_d[:])
            s_all = cpool.tile([128, NT], dt.float32)
            em_all = cpool.tile([128, NT], dt.float32)

            for t in range(NT):
                # encoder tile, feature-major chunks: free = [c, h]
                enc_t = enc_pool.tile([128, 512], dt.float32)
                nc.gpsimd.dma_start(
                    enc_t[:].rearrange("p (c h) -> p c h", c=4),
                    encT_d[:, :, ts(t, 128)].rearrange("c p h -> p c h"),
                )
                # gather-transpose T0[tok0], T1[tok1] -> (128, 4, 128) bf16
                g0 = g_pool.tile([128, 4, 128], dt.bfloat16, tag="g0")
                g1 = g_pool.tile([128, 4, 128], dt.bfloat16, tag="g1")
                nc.gpsimd.dma_gather(
                    g0[:], t0_d[:], idx_sb[:, t * 16: t * 16 + 8],
                    num_idxs=128, num_idxs_reg=128, elem_size=512, transpose=True,
                )
                nc.gpsimd.dma_gather(
                    g1[:], t1_d[:], idx_sb[:, t * 16 + 8: t * 16 + 16],
                    num_idxs=128, num_idxs_reg=128, elem_size=512, transpose=True,
                )
                g0f = g0[:].rearrange("p c h -> p (c h)")
                g1f = g1[:].rearrange("p c h -> p (c h)")
                dsum = w_pool.tile([128, 512], dt.bfloat16, tag="dsum")
                nc.vector.tensor_add(dsum[:], g0f, g1f)
                decT = w_pool.tile([128, 512], dt.bfloat16, tag="decT")
                nc.vector.tensor_scalar_max(decT[:], dsum[:], 0.0)

                # proj: PT[jc] += pwT[dc,jc]^T @ decT[dc]  (bf16, feature-major out)
                pt_ps = pt_pool.tile([128, 512], dt.float32)
                for jc in range(4):
                    for dc in range(4):
                        nc.tensor.matmul(
                            pt_ps[:, ts(jc, 128)],
                            pwT_sb[:, dc * 512 + jc * 128: dc * 512 + (jc + 1) * 128],
                            decT[:, ts(dc, 128)],
                            start=(dc == 0), stop=(dc == 3),
                        )

                a_pre = w_pool.tile([128, 512], dt.float32, tag="a_pre")
                nc.vector.tensor_add(a_pre[:], pt_ps[:], enc_t[:])
                at = w_pool.tile([128, 512], dt.float32r, tag="at")
                nc.scalar.activation(at[:], a_pre[:], mybir.ActivationFunctionType.Tanh)

                # joiner: logits[h, v] = sum_j AT[j, h] * jwT[j, v] + jb[v]
                lg_ps = lg_pool.tile([128, 500], dt.float32)
                nc.tensor.matmul(
                    lg_ps[:], ones_sb[:], jb_sb[:],
                    start=True, stop=False,
                )
                for jc in range(4):
                    nc.tensor.matmul(
                        lg_ps[:],
                        at[:, ts(jc, 128)],
                        jwT_sb[:, jc * 500: (jc + 1) * 500],
                        start=False, stop=(jc == 3),
                    )

                exp_sb = w_pool.tile([128, 500], dt.float32, tag="exp")
                nc.scalar.activation(
                    exp_sb[:], lg_ps[:], mybir.ActivationFunctionType.Exp,
                    accum_out=s_all[:, t: t + 1],
                )
                nc.vector.tensor_reduce(
                    em_all[:, t: t + 1], exp_sb[:],
                    axis=mybir.AxisListType.X, op=mybir.AluOpType.max,
                )

                if debug_tile is not None and t == debug_tile:
                    nc.gpsimd.dma_start(dbg["decT"][:], decT[:])
                    nc.gpsimd.dma_start(dbg["at"][:], at[:])
                    nc.gpsimd.dma_start(dbg["logits"][:], lg_ps[:])

            nc.gpsimd.dma_start(s_d[:], s_all[:])
            nc.gpsimd.dma_start(em_d[:], em_all[:])

    nc.finalize()
    return nc


def _host_prep(inputs):
    import ml_dtypes

    di = np.asarray(inputs["decoder_input"])
    enc = np.asarray(inputs["encoder_out"], dtype=np.float32)
    emb = np.asarray(inputs["embed_table"], dtype=np.float32)
    cw = np.asarray(inputs["conv_w"], dtype=np.float32)
    pw = np.asarray(inputs["proj_w"], dtype=np.float32)
    pb = np.asarray(inputs["proj_b"], dtype=np.float32)
    jw = np.asarray(inputs["joiner_w"], dtype=np.float32)
    jb = np.asarray(inputs["joiner_b"], dtype=np.float32)

    bf16 = ml_dtypes.bfloat16
    g = np.arange(DEC_DIM) // 4
    # T_k[v, o] = sum_i emb[v, 4g(o)+i] * cw[o, i, k]
    T0 = np.zeros((VOCAB, DEC_DIM), np.float32)
    T1 = np.zeros((VOCAB, DEC_DIM), np.float32)
    for i in range(4):
        T0 += emb[:, 4 * g + i] * cw[:, i, 0]
        T1 += emb[:, 4 * g + i] * cw[:, i, 1]
    t0_b = T0.astype(bf16)
    t1_b = T1.astype(bf16)

    # pwT_sb[p, dc*512 + j] = pw[j, dc*128 + p]
    pwT = np.empty((128, 4 * 512), np.float32)
    for dc in range(4):
        pwT[:, dc * 512:(dc + 1) * 512] = pw[:, dc * 128:(dc + 1) * 128].T
    pwT_b = pwT.astype(bf16)
    # jwT_sb[p, jc*500 + v] = jw[v, jc*128 + p]
    jwT = np.empty((128, 4 * 500), np.float32)
    for jc in range(4):
        jwT[:, jc * 500:(jc + 1) * 500] = jw[:, jc * 128:(jc + 1) * 128].T

    in_maps = []
    for c in range(NCORES):
        lo = c * NLOC
        enc_s = enc[lo: lo + NLOC] + pb[None, :]          # fold proj_b
        # encT[cc, p, h] = enc_s[h, cc*128 + p]
        encT = np.ascontiguousarray(
            enc_s.T.reshape(4, 128, NLOC)
        )
        tok = di[lo: lo + NLOC]                            # (NLOC, 2)
        idx = np.zeros((128, NT * 16), np.int16)
        for t in range(NT):
            blk = tok[t * 128:(t + 1) * 128]               # (128, 2)
            # unwrapped[i] = idx[i % 16, i // 16]
            idx[:16, t * 16: t * 16 + 8] = blk[:, 0].reshape(8, 16).T
            idx[:16, t * 16 + 8: t * 16 + 16] = blk[:, 1].reshape(8, 16).T
        in_maps.append({
            "t0": np.asarray(t0_b), "t1": np.asarray(t1_b),
            "encT": encT, "idx": idx,
            "pwT": np.asarray(pwT_b), "jwT": jwT,
            "jb": jb.reshape(1, 500),
            "ones": np.ones((1, 128), np.float32),
        })
    aux = {"T0": T0, "T1": T1}
    return in_maps, aux


def _host_finish(inputs, s_list, em_list):
    """Rank rows by device stats, recompute top rows exactly, global top-k."""
    di = np.asarray(inputs["decoder_input"])
    enc = np.asarray(inputs["encoder_out"], dtype=np.float32)
    hlp = np.asarray(inputs["hyps_log_prob"], dtype=np.float32).reshape(-1)
    emb = np.asarray(inputs["embed_table"], dtype=np.float32)
    cw = np.asarray(inputs["conv_w"], dtype=np.float32)
    pw = np.asarray(inputs["proj_w"], dtype=np.float32)
    pb = np.asarray(inputs["proj_b"], dtype=np.float32)
    jw = np.asarray(inputs["joiner_w"], dtype=np.float32)
    jb = np.asarray(inputs["joiner_b"], dtype=np.float32)
    beam = int(np.asarray(inputs["beam"]))

    # device stats -> rowM = hlp + log(max_exp) - log(sum_exp)
    rowM = np.empty(NUM_HYPS, np.float64)
    for c in range(NCORES):
        s = s_list[c].astype(np.float64)      # (128, NT)
        em = em_list[c].astype(np.float64)
        # row (p, t) -> hyp c*NLOC + t*128 + p
        rm = np.log(em) - np.log(s)           # (128, NT)
        rowM[c * NLOC:(c + 1) * NLOC] = rm.T.reshape(-1)
    rowM += hlp

    rows = np.argsort(-rowM)[:TOPROWS].astype(np.int64)

    # exact f32 recompute of the selected rows (mirrors the reference)
    g = np.arange(DEC_DIM) // 4
    tok = di[rows]                                         # (R, 2)
    embg = emb[np.clip(tok, 0, None)]                      # (R, 2, 512)
    embg = embg * (tok >= 0)[..., None].astype(np.float32)
    x = np.zeros((len(rows), DEC_DIM), np.float32)
    for i in range(4):
        x += embg[:, 0, 4 * g + i] * cw[:, i, 0] + embg[:, 1, 4 * g + i] * cw[:, i, 1]
    dec = np.maximum(x, 0.0)
    P = dec @ pw.T + pb
    A = np.tanh(enc[rows] + P)
    logits = A @ jw.T + jb
    m = logits.max(1, keepdims=True)
    lse = m + np.log(np.exp(logits - m).sum(1, keepdims=True))
    tlp = logits - lse                                     # (R, 500)
    lp = tlp + hlp[rows, None]

    flat = lp.reshape(-1)
    ordloc = np.argsort(-flat)[:beam]
    r_i, t_i = ordloc // VOCAB, ordloc % VOCAB
    hyp_idx = rows[r_i].astype(np.int32)
    tok_idx = t_i.astype(np.int32)
    vals = flat[ordloc].astype(np.float32)
    tok_prob = np.exp(tlp[r_i, t_i]).astype(np.float32)
    return vals, tok_prob, hyp_idx, tok_idx


def kernel(**inputs):
    from concourse.bass_utils import run_bass_kernel_spmd

    if "nc" not in _CACHE:
        _CACHE["nc"] = _build_program()
    nc = _CACHE["nc"]
    in_maps, _ = _host_prep(inputs)
    res = run_bass_kernel_spmd(nc, in_maps, list(range(NCORES)))
    s_list = [res.results[c]["s_out"] for c in range(NCORES)]
    em_list = [res.results[c]["em_out"] for c in range(NCORES)]
    return _host_finish(inputs, s_list, em_list)


# revision 4
# speedup vs baseline: 1.2212x; 1.2212x over previous
"""Trainium2 Bass kernel for nn_DecoderModule (topk_masking).

Strategy: data-parallel over num_hyps across 8 NeuronCores. Each core
computes, for its 8192-hyp shard, per-row softmax statistics
(sumexp(logits) and max(exp(logits))) of the joiner logits. The host
then ranks rows by rowM = hyps_log_prob + log(max_exp) - log(sum_exp)
(exactly the per-row max of the final log-probs), recomputes the top
candidate rows exactly in f32, and takes the global top-k. This is the
"per-shard top-k + all-gather + global top-k" scheme with the per-shard
top-k expressed as per-row stats (a row can contribute up to beam=4
candidates, so the top-4 rows by row-max are a guaranteed superset).

Device pipeline per 128-hyp tile (gathers batched GB tiles per group):
  - dma_gather(transpose=True) pulls conv-folded embedding table rows
    (T0[tok0], T1[tok1]) already transposed to feature-major bf16
  - DVE add + relu -> decT (bf16)
  - 16 bf16 matmuls: PT = proj_w^T-chunks @ decT-chunks (feature-major)
  - DVE add with host-pretransposed encoder (proj_b folded in)
  - ScalarE tanh -> AT
  - 4 f32r matmuls + 1 bias matmul -> logits (PSUM)
  - ScalarE Exp with accum_out -> sumexp per row; DVE reduce-max of exp
"""

import numpy as np

NUM_HYPS = 65536
VOCAB = 500
DEC_DIM = 512
JOINER_DIM = 512
CTX = 2
NCORES = 8
NLOC = NUM_HYPS // NCORES          # 8192 hyps per core
NT = NLOC // 128                   # 64 tiles per core
GB = 4                             # tiles per gather group
NG = NT // GB                      # gather groups
TOPROWS = 16                       # rows recomputed exactly on host

_CACHE = {}


def _build_program(debug_tile=None):
    import concourse.bacc as bacc
    import concourse.mybir as mybir
    from concourse.tile import TileContext
    from concourse.bass import ts

    dt = mybir.dt
    nc = bacc.Bacc("TRN2", debug=False, num_devices=NCORES)

    t0_d = nc.dram_tensor("t0", [VOCAB, DEC_DIM], dt.bfloat16, kind="ExternalInput")
    t1_d = nc.dram_tensor("t1", [VOCAB, DEC_DIM], dt.bfloat16, kind="ExternalInput")
    encT_d = nc.dram_tensor("encT", [4, 128, NLOC], dt.float32, kind="ExternalInput")
    idx_d = nc.dram_tensor("idx", [128, NG * 64], dt.int16, kind="ExternalInput")
    pwT_d = nc.dram_tensor("pwT", [128, 4 * 512], dt.bfloat16, kind="ExternalInput")
    jwT_d = nc.dram_tensor("jwT", [128, 4 * 500], dt.float32r, kind="ExternalInput")
    jb_d = nc.dram_tensor("jb", [1, 500], dt.float32r, kind="ExternalInput")
    ones_d = nc.dram_tensor("ones", [1, 128], dt.float32r, kind="ExternalInput")
    s_d = nc.dram_tensor("s_out", [128, NT], dt.float32, kind="ExternalOutput")
    em_d = nc.dram_tensor("em_out", [128, NT], dt.float32, kind="ExternalOutput")
    dbg = {}
    if debug_tile is not None:
        dbg["decT"] = nc.dram_tensor("dbg_decT", [128, 512], dt.bfloat16, kind="ExternalOutput")
        dbg["at"] = nc.dram_tensor("dbg_at", [128, 512], dt.float32, kind="ExternalOutput")
        dbg["logits"] = nc.dram_tensor("dbg_logits", [128, 500], dt.float32, kind="ExternalOutput")

    with TileContext(nc) as tc:
        with (
            tc.tile_pool(name="consts", bufs=1) as cpool,
            tc.tile_pool(name="enc", bufs=4) as enc_pool,
            tc.tile_pool(name="gather", bufs=2) as g_pool,
            tc.tile_pool(name="dec", bufs=2) as d_pool,
            tc.tile_pool(name="work", bufs=3) as w_pool,
            tc.tile_pool(name="psum_pt", bufs=2, space="PSUM") as pt_pool,
            tc.tile_pool(name="psum_lg", bufs=2, space="PSUM") as lg_pool,
        ):
            idx_sb = cpool.tile([128, NG * 64], dt.int16)
            nc.sync.dma_start(idx_sb[:], idx_d[:])
            pwT_sb = cpool.tile([128, 4 * 512], dt.bfloat16)
            nc.sync.dma_start(pwT_sb[:], pwT_d[:])
            jwT_sb = cpool.tile([128, 4 * 500], dt.float32r)
            nc.sync.dma_start(jwT_sb[:], jwT_d[:])
            jb_sb = cpool.tile([1, 500], dt.float32r)
            nc.sync.dma_start(jb_sb[:], jb_d[:])
            ones_sb = cpool.tile([1, 128], dt.float32r)
            nc.sync.dma_start(ones_sb[:], ones_d[:])
            s_all = cpool.tile([128, NT], dt.float32)
            em_all = cpool.tile([128, NT], dt.float32)

            for grp in range(NG):
                # gather-transpose T0[tok0], T1[tok1] for GB tiles at once
                g0 = g_pool.tile([128, 4, GB * 128], dt.bfloat16, tag="g0")
                g1 = g_pool.tile([128, 4, GB * 128], dt.bfloat16, tag="g1")
                nc.gpsimd.dma_gather(
                    g0[:], t0_d[:], idx_sb[:, grp * 64: grp * 64 + 32],
                    num_idxs=GB * 128, num_idxs_reg=GB * 128,
                    elem_size=512, transpose=True,
                )
                nc.gpsimd.dma_gather(
                    g1[:], t1_d[:], idx_sb[:, grp * 64 + 32: grp * 64 + 64],
                    num_idxs=GB * 128, num_idxs_reg=GB * 128,
                    elem_size=512, transpose=True,
                )
                g0f = g0[:].rearrange("p c h -> p (c h)")
                g1f = g1[:].rearrange("p c h -> p (c h)")
                dsum = d_pool.tile([128, 4 * GB * 128], dt.bfloat16, tag="dsum")
                nc.vector.tensor_add(dsum[:], g0f, g1f)
                decT = d_pool.tile([128, 4, GB * 128], dt.bfloat16, tag="decT")
                nc.vector.tensor_scalar_max(
                    decT[:].rearrange("p c h -> p (c h)"), dsum[:], 0.0)

                for j in range(GB):
                    t = grp * GB + j
                    # encoder tile, feature-major chunks: free = [c, h]
                    enc_t = enc_pool.tile([128, 512], dt.float32)
                    nc.sync.dma_start(
                        enc_t[:].rearrange("p (c h) -> p c h", c=4),
                        encT_d[:, :, ts(t, 128)].rearrange("c p h -> p c h"),
                    )

                    # proj: PT[jc] += pwT[dc,jc]^T @ decT[dc] (bf16, feat-major)
                    pt_ps = pt_pool.tile([128, 512], dt.float32)
                    for jc in range(4):
                        for dc in range(4):
                            nc.tensor.matmul(
                                pt_ps[:, ts(jc, 128)],
                                pwT_sb[:, dc * 512 + jc * 128: dc * 512 + (jc + 1) * 128],
                                decT[:, dc, j * 128:(j + 1) * 128],
                                start=(dc == 0), stop=(dc == 3),
                            )

                    a_pre = w_pool.tile([128, 512], dt.float32, tag="a_pre")
                    nc.vector.tensor_add(a_pre[:], pt_ps[:], enc_t[:])
                    at = w_pool.tile([128, 512], dt.float32r, tag="at")
                    nc.scalar.activation(at[:], a_pre[:], mybir.ActivationFunctionType.Tanh)

                    # joiner: logits[h, v] = sum_j AT[j, h] * jwT[j, v] + jb[v]
                    lg_ps = lg_pool.tile([128, 500], dt.float32)
                    nc.tensor.matmul(lg_ps[:], ones_sb[:], jb_sb[:], start=True, stop=False)
                    for jc in range(4):
                        nc.tensor.matmul(
                            lg_ps[:],
                            at[:, ts(jc, 128)],
                            jwT_sb[:, jc * 500: (jc + 1) * 500],
                            start=False, stop=(jc == 3),
                        )

                    exp_sb = w_pool.tile([128, 500], dt.float32, tag="exp")
                    nc.scalar.activation(
                        exp_sb[:], lg_ps[:], mybir.ActivationFunctionType.Exp,
                        accum_out=s_all[:, t: t + 1],
                    )
                    nc.vector.tensor_reduce(
                        em_all[:, t: t + 1], exp_sb[:],
                        axis=mybir.AxisListType.X, op=mybir.AluOpType.max,
                    )

                    if debug_tile is not None and t == debug_tile:
                        nc.gpsimd.dma_start(dbg["decT"][:], decT[:, :, j * 128:(j + 1) * 128])
                        nc.gpsimd.dma_start(dbg["at"][:], at[:])
                        nc.gpsimd.dma_start(dbg["logits"][:], lg_ps[:])

            nc.sync.dma_start(s_d[:], s_all[:])
            nc.sync.dma_start(em_d[:], em_all[:])

    nc.finalize()
    return nc


def _host_prep(inputs):
    import ml_dtypes

    di = np.asarray(inputs["decoder_input"])
    enc = np.asarray(inputs["encoder_out"], dtype=np.float32)
    emb = np.asarray(inputs["embed_table"], dtype=np.float32)
    cw = np.asarray(inputs["conv_w"], dtype=np.float32)
    pw = np.asarray(inputs["proj_w"], dtype=np.float32)
    pb = np.asarray(inputs["proj_b"], dtype=np.float32)
    jw = np.asarray(inputs["joiner_w"], dtype=np.float32)
    jb = np.asarray(inputs["joiner_b"], dtype=np.float32)

    bf16 = ml_dtypes.bfloat16
    g = np.arange(DEC_DIM) // 4
    # T_k[v, o] = sum_i emb[v, 4g(o)+i] * cw[o, i, k]
    T0 = np.zeros((VOCAB, DEC_DIM), np.float32)
    T1 = np.zeros((VOCAB, DEC_DIM), np.float32)
    for i in range(4):
        T0 += emb[:, 4 * g + i] * cw[:, i, 0]
        T1 += emb[:, 4 * g + i] * cw[:, i, 1]
    t0_b = T0.astype(bf16)
    t1_b = T1.astype(bf16)

    # pwT_sb[p, dc*512 + j] = pw[j, dc*128 + p]
    pwT = np.empty((128, 4 * 512), np.float32)
    for dc in range(4):
        pwT[:, dc * 512:(dc + 1) * 512] = pw[:, dc * 128:(dc + 1) * 128].T
    pwT_b = pwT.astype(bf16)
    # jwT_sb[p, jc*500 + v] = jw[v, jc*128 + p]
    jwT = np.empty((128, 4 * 500), np.float32)
    for jc in range(4):
        jwT[:, jc * 500:(jc + 1) * 500] = jw[:, jc * 128:(jc + 1) * 128].T

    in_maps = []
    for c in range(NCORES):
        lo = c * NLOC
        enc_s = enc[lo: lo + NLOC] + pb[None, :]          # fold proj_b
        # encT[cc, p, h] = enc_s[h, cc*128 + p]
        encT = np.ascontiguousarray(enc_s.T.reshape(4, 128, NLOC))
        tok = di[lo: lo + NLOC]                            # (NLOC, 2)
        # gather idx layout per group: unwrapped[i] = idx[i % 16, i // 16]
        idx = np.zeros((128, NG * 64), np.int16)
        for grp in range(NG):
            blk = tok[grp * GB * 128:(grp + 1) * GB * 128]   # (GB*128, 2)
            idx[:16, grp * 64: grp * 64 + 32] = blk[:, 0].reshape(32, 16).T
            idx[:16, grp * 64 + 32: grp * 64 + 64] = blk[:, 1].reshape(32, 16).T
        in_maps.append({
            "t0": np.asarray(t0_b), "t1": np.asarray(t1_b),
            "encT": encT, "idx": idx,
            "pwT": np.asarray(pwT_b), "jwT": jwT,
            "jb": jb.reshape(1, 500),
            "ones": np.ones((1, 128), np.float32),
        })
    aux = {"T0": T0, "T1": T1}
    return in_maps, aux


def _host_finish(inputs, s_list, em_list):
    """Rank rows by device stats, recompute top rows exactly, global top-k."""
    di = np.asarray(inputs["decoder_input"])
    enc = np.asarray(inputs["encoder_out"], dtype=np.float32)
    hlp = np.asarray(inputs["hyps_log_prob"], dtype=np.float32).reshape(-1)
    emb = np.asarray(inputs["embed_table"], dtype=np.float32)
    cw = np.asarray(inputs["conv_w"], dtype=np.float32)
    pw = np.asarray(inputs["proj_w"], dtype=np.float32)
    pb = np.asarray(inputs["proj_b"], dtype=np.float32)
    jw = np.asarray(inputs["joiner_w"], dtype=np.float32)
    jb = np.asarray(inputs["joiner_b"], dtype=np.float32)
    beam = int(np.asarray(inputs["beam"]))

    # device stats -> rowM = hlp + log(max_exp) - log(sum_exp)
    rowM = np.empty(NUM_HYPS, np.float64)
    for c in range(NCORES):
        s = s_list[c].astype(np.float64)      # (128, NT)
        em = em_list[c].astype(np.float64)
        # row (p, t) -> hyp c*NLOC + t*128 + p
        rm = np.log(em) - np.log(s)           # (128, NT)
        rowM[c * NLOC:(c + 1) * NLOC] = rm.T.reshape(-1)
    rowM += hlp

    rows = np.argsort(-rowM)[:TOPROWS].astype(np.int64)

    # exact f32 recompute of the selected rows (mirrors the reference)
    g = np.arange(DEC_DIM) // 4
    tok = di[rows]                                         # (R, 2)
    embg = emb[np.clip(tok, 0, None)]                      # (R, 2, 512)
    embg = embg * (tok >= 0)[..., None].astype(np.float32)
    x = np.zeros((len(rows), DEC_DIM), np.float32)
    for i in range(4):
        x += embg[:, 0, 4 * g + i] * cw[:, i, 0] + embg[:, 1, 4 * g + i] * cw[:, i, 1]
    dec = np.maximum(x, 0.0)
    P = dec @ pw.T + pb
    A = np.tanh(enc[rows] + P)
    logits = A @ jw.T + jb
    m = logits.max(1, keepdims=True)
    lse = m + np.log(np.exp(logits - m).sum(1, keepdims=True))
    tlp = logits - lse                                     # (R, 500)
    lp = tlp + hlp[rows, None]

    flat = lp.reshape(-1)
    ordloc = np.argsort(-flat)[:beam]
    r_i, t_i = ordloc // VOCAB, ordloc % VOCAB
    hyp_idx = rows[r_i].astype(np.int32)
    tok_idx = t_i.astype(np.int32)
    vals = flat[ordloc].astype(np.float32)
    tok_prob = np.exp(tlp[r_i, t_i]).astype(np.float32)
    return vals, tok_prob, hyp_idx, tok_idx


def kernel(**inputs):
    from concourse.bass_utils import run_bass_kernel_spmd

    if "nc" not in _CACHE:
        _CACHE["nc"] = _build_program()
    nc = _CACHE["nc"]
    in_maps, _ = _host_prep(inputs)
    res = run_bass_kernel_spmd(nc, in_maps, list(range(NCORES)))
    s_list = [res.results[c]["s_out"] for c in range(NCORES)]
    em_list = [res.results[c]["em_out"] for c in range(NCORES)]
    return _host_finish(inputs, s_list, em_list)
